# revision 2
# baseline (speedup 1.0000x reference)
"""BP-MLL loss kernel for Trainium2 (Bass/Tile), data-parallel over 8 NeuronCores.

Reference computation (per row r of [B, L] inputs):
    s_pos[r] = sum_{j: t=1} exp(-x[r,j])
    s_neg[r] = sum_{j: t=0} exp( x[r,j])
    n_pos[r] = #{j: t=1},  n_neg[r] = L - n_pos[r]
    loss     = sum_r s_pos[r]*s_neg[r] / (n_pos[r]*n_neg[r])

Sharding: batch dim B=8192 split 8 ways (1024 rows/core). Each core streams its
[1024, 10000] x/t slabs once and emits per-(row, column-chunk) partial sums
(s_pos, s_neg, su); the tiny per-row combine (exact n_pos recovery, product,
divide, global sum) runs on host in float64.

Per-core device plan. The 0/1 mask is folded into the exp arguments so each
tile [128 rows, w cols] (rows on partitions) needs one DVE pass and two ACT
passes, each with a fused free-axis accumulation:
    DVE:  u = C*t - x  (C = 8192 = 2^13)      accum -> su = C*n_pos - sum(x)
    ACT:  exp(u - C) = exp(-x) if t=1 else 0  accum -> s_pos
    ACT:  exp(-u)    = exp(x)  if t=0 else 0  accum -> s_neg
C is a power of 2, so C*t is exact; |sum(x)| + rounding noise in su is O(300)
<< C/2, so round(su/C) on host recovers n_pos exactly. fl(C - x) costs x half
an ulp of C (2^-11): ~1e-5 relative noise in s_pos, zero-mean across a row.
exp(-C...) flushes cleanly to 0.

Performance shape (trace-driven): the x/t stream is DMA-bound at ~421 GB/s
(HBM/fabric ceiling) and gapless; ACT runs at ~85% mid-stream. The last row
group's column chunks taper geometrically so the post-stream serial tail
(last stt -> 2 exps -> out-DMA) is ~2-3 us instead of ~10. All per-chunk
accumulator slots live in one [128, 3*n_slots] SBUF tile and leave in a single
DMA at the end; there is no on-device epilogue (no reduce/recip/matmul tail).
"""

import numpy as np

import concourse.bacc as bacc
import concourse.bass as bass
import concourse.tile as tile
from concourse import mybir
from concourse.bass_utils import run_bass_kernel_spmd

F32 = mybir.dt.float32
I32 = mybir.dt.int32
AF = mybir.ActivationFunctionType
ALU = mybir.AluOpType

B, L = 8192, 10000
N_CORES = 8
ROWS = B // N_CORES  # rows per core
P = 128
N_RG = ROWS // P  # row groups per core
BIG = 8192.0  # mask scale: power of 2; exp(-8192) flushes to 0,
# and n_pos = round(su/BIG) is exact since |sum(x)| << BIG/2

F_C = 2500  # full chunk width
# Last row group tapers so the end-of-stream compute tail is short.
TAPER = (2500, 2500, 1250, 1250, 1000, 750, 500, 250)
assert sum(TAPER) == L


def _chunks_for(rg):
    """Per row group: list of (col_offset, width)."""
    ws = list(TAPER) if rg == N_RG - 1 else [F_C] * (L // F_C)
    offs = np.concatenate([[0], np.cumsum(ws)[:-1]]).tolist()
    return list(zip(offs, ws))


N_SLOTS = sum(len(_chunks_for(rg)) for rg in range(N_RG))


def build_bass(io_bufs=6, u_bufs=4, dma_only=False):
    """Build the per-core Bass program. Same program runs SPMD on all cores."""
    nc = bacc.Bacc("TRN2", target_bir_lowering=False, debug=False)
    x = nc.dram_tensor("x", [ROWS, L], F32, kind="ExternalInput").ap()
    t = nc.dram_tensor("t", [ROWS, L], I32, kind="ExternalInput").ap()
    out = nc.dram_tensor("out", [P, 3 * N_SLOTS], F32, kind="ExternalOutput").ap()

    with tile.TileContext(nc) as tc:
        with (
            tc.tile_pool(name="io", bufs=io_bufs) as io_pool,
            tc.tile_pool(name="upool", bufs=u_bufs) as u_pool,
            tc.tile_pool(name="epool", bufs=2) as e_pool,
            tc.tile_pool(name="acc", bufs=1) as acc_pool,
        ):
            # acc columns: [0,N) s_pos | [N,2N) s_neg | [2N,3N) su
            acc = acc_pool.tile([P, 3 * N_SLOTS], F32, tag="acc")
            if not dma_only:
                neg_big = acc_pool.tile([P, 1], F32, tag="neg_big")
                nc.vector.memset(neg_big[:], -BIG)

            sl = 0
            for rg in range(N_RG):
                r0 = rg * P
                for c0, fw in _chunks_for(rg):
                    xt = io_pool.tile([P, fw], F32, tag="x")
                    tt = io_pool.tile([P, fw], I32, tag="t")
                    nc.sync.dma_start(xt[:], x[r0 : r0 + P, c0 : c0 + fw])
                    nc.sync.dma_start(tt[:], t[r0 : r0 + P, c0 : c0 + fw])
                    if dma_only:
                        sl += 1
                        continue

                    ut = u_pool.tile([P, fw], F32, tag="u")
                    # u = C*t - x ; accum -> su = C*n_pos - sum(x)
                    nc.vector.scalar_tensor_tensor(
                        ut[:],
                        tt[:],
                        BIG,
                        xt[:],
                        op0=ALU.mult,
                        op1=ALU.subtract,
                        accum_out=acc[:, 2 * N_SLOTS + sl : 2 * N_SLOTS + sl + 1],
                    )
                    ea = e_pool.tile([P, fw], F32, tag="escr")
                    # exp(u - C): t=1 -> exp(-x); t=0 -> 0
                    nc.scalar.activation(
                        ea[:],
                        ut[:],
                        AF.Exp,
                        bias=neg_big[:],
                        scale=1.0,
                        accum_out=acc[:, sl : sl + 1],
                    )
                    eb = e_pool.tile([P, fw], F32, tag="escr")
                    # exp(-u): t=0 -> exp(x); t=1 -> 0
                    nc.scalar.activation(
                        eb[:],
                        ut[:],
                        AF.Exp,
                        scale=-1.0,
                        accum_out=acc[:, N_SLOTS + sl : N_SLOTS + sl + 1],
                    )
                    sl += 1

            if dma_only:
                nc.vector.memset(acc[:, 0:1], 0.0)
            # Single out-DMA, issued from the scalar engine: it owns the last
            # accumulator write, so no cross-engine hop on the critical tail.
            nc.scalar.dma_start(out[:, :], acc[:])

    nc.compile()
    return nc


_NC_CACHE = {}


def _get_nc():
    if "nc" not in _NC_CACHE:
        _NC_CACHE["nc"] = build_bass()
    return _NC_CACHE["nc"]


def _shard_inputs(x, t):
    return [
        {
            "x": np.ascontiguousarray(x[i * ROWS : (i + 1) * ROWS]),
            "t": np.ascontiguousarray(t[i * ROWS : (i + 1) * ROWS]),
        }
        for i in range(N_CORES)
    ]


# slot -> row group, precomputed once
_SLOT_RG = np.array(
    [rg for rg in range(N_RG) for _ in _chunks_for(rg)], dtype=np.int64
)


def _combine(core_outs):
    """Host epilogue: core_outs is a list of [P, 3*N_SLOTS] f32 arrays."""
    total = 0.0
    for o in core_outs:
        o = np.asarray(o, dtype=np.float64)
        spos_sl = o[:, 0:N_SLOTS]
        sneg_sl = o[:, N_SLOTS : 2 * N_SLOTS]
        su_sl = o[:, 2 * N_SLOTS : 3 * N_SLOTS]
        # per-row sums over this row group's slots: result [P, N_RG]
        s_pos = np.stack(
            [spos_sl[:, _SLOT_RG == rg].sum(axis=1) for rg in range(N_RG)], axis=1
        )
        s_neg = np.stack(
            [sneg_sl[:, _SLOT_RG == rg].sum(axis=1) for rg in range(N_RG)], axis=1
        )
        su = np.stack(
            [su_sl[:, _SLOT_RG == rg].sum(axis=1) for rg in range(N_RG)], axis=1
        )
        n_pos = np.rint(su / BIG)  # exact: |sum(x) + eps| << BIG/2
        n_neg = float(L) - n_pos
        total += float((s_pos * s_neg / (n_pos * n_neg)).sum())
    return np.float32(total)


def kernel(input, target):
    x = np.ascontiguousarray(np.asarray(input, dtype=np.float32))
    t = np.ascontiguousarray(np.asarray(target, dtype=np.int32))
    assert x.shape == (B, L) and t.shape == (B, L)

    nc = _get_nc()
    res = run_bass_kernel_spmd(
        nc, _shard_inputs(x, t), core_ids=list(range(N_CORES))
    )
    return _combine([res.results[i]["out"] for i in range(N_CORES)])


# revision 3
# speedup vs baseline: 1.0475x; 1.0475x over previous
"""BP-MLL loss kernel for Trainium2 (Bass/Tile), data-parallel over 8 NeuronCores.

Reference computation (per row r of [B, L] inputs):
    s_pos[r] = sum_{j: t=1} exp(-x[r,j])
    s_neg[r] = sum_{j: t=0} exp( x[r,j])
    n_pos[r] = #{j: t=1},  n_neg[r] = L - n_pos[r]
    loss     = sum_r s_pos[r]*s_neg[r] / (n_pos[r]*n_neg[r])

Sharding: batch dim B=8192 split 8 ways (1024 rows/core). Each core streams its
[1024, 10000] x/t slabs once and emits per-(row, column-chunk) partial sums
(s_pos, s_neg, su); the tiny per-row combine (exact n_pos recovery, product,
divide, global sum) runs on host in float64.

Per-core device plan. The 0/1 mask is folded into the exp arguments so each
tile [128 rows, w cols] (rows on partitions) needs one DVE pass and two ACT
passes, each with a fused free-axis accumulation:
    DVE:  u = C*t - x  (C = 8192 = 2^13)      accum -> su = C*n_pos - sum(x)
    ACT:  exp(u - C) = exp(-x) if t=1 else 0  accum -> s_pos
    ACT:  exp(-u)    = exp(x)  if t=0 else 0  accum -> s_neg
C is a power of 2, so C*t is exact; |sum(x)| + rounding noise in su is O(300)
<< C/2, so round(su/C) on host recovers n_pos exactly. fl(C - x) costs x half
an ulp of C (2^-11): ~1e-5 relative noise in s_pos, zero-mean across a row.
exp(-C...) flushes cleanly to 0.

Performance shape (trace-driven): the x/t stream is DMA-bound at ~421 GB/s
(HBM/fabric ceiling) and gapless; ACT runs at ~85% mid-stream. The last row
group's column chunks taper geometrically so the post-stream serial tail
(last stt -> 2 exps -> out-DMA) is ~2-3 us instead of ~10. All per-chunk
accumulator slots live in one [128, 3*n_slots] SBUF tile and leave in a single
DMA at the end; there is no on-device epilogue (no reduce/recip/matmul tail).
"""

import numpy as np

import concourse.bacc as bacc
import concourse.bass as bass
import concourse.tile as tile
from concourse import mybir
from concourse.bass_utils import run_bass_kernel_spmd

F32 = mybir.dt.float32
I32 = mybir.dt.int32
AF = mybir.ActivationFunctionType
ALU = mybir.AluOpType

B, L = 8192, 10000
N_CORES = 8
ROWS = B // N_CORES  # rows per core
P = 128
N_RG = ROWS // P  # row groups per core
BIG = 8192.0  # mask scale: power of 2; exp(-8192) flushes to 0,
# and n_pos = round(su/BIG) is exact since |sum(x)| << BIG/2

F_C = 2500  # full chunk width
# Last row group tapers so the end-of-stream compute tail is short.
TAPER = (2500, 2500, 1250, 1250, 1000, 750, 500, 250)
assert sum(TAPER) == L


def _chunks_for(rg):
    """Per row group: list of (col_offset, width)."""
    ws = list(TAPER) if rg == N_RG - 1 else [F_C] * (L // F_C)
    offs = np.concatenate([[0], np.cumsum(ws)[:-1]]).tolist()
    return list(zip(offs, ws))


N_SLOTS = sum(len(_chunks_for(rg)) for rg in range(N_RG))


def build_bass(io_bufs=6, u_bufs=4, dma_only=False):
    """Build the per-core Bass program. Same program runs SPMD on all cores."""
    nc = bacc.Bacc("TRN2", target_bir_lowering=False, debug=False)
    x = nc.dram_tensor("x", [ROWS, L], F32, kind="ExternalInput").ap()
    t = nc.dram_tensor("t", [ROWS, L], I32, kind="ExternalInput").ap()
    out = nc.dram_tensor("out", [P, 3 * N_SLOTS], F32, kind="ExternalOutput").ap()

    with tile.TileContext(nc) as tc:
        with (
            tc.tile_pool(name="io", bufs=io_bufs) as io_pool,
            tc.tile_pool(name="upool", bufs=u_bufs) as u_pool,
            tc.tile_pool(name="epool", bufs=2) as e_pool,
            tc.tile_pool(name="acc", bufs=1) as acc_pool,
        ):
            # One accumulator tile per (engine, kind): a tile written by two
            # different engines serializes them (coarse cross-engine dep
            # tracking), which stalls the whole stream.
            acc_spos = acc_pool.tile([P, N_SLOTS], F32, tag="acc_spos")
            acc_sneg = acc_pool.tile([P, N_SLOTS], F32, tag="acc_sneg")
            acc_su = acc_pool.tile([P, N_SLOTS], F32, tag="acc_su")
            if not dma_only:
                neg_big = acc_pool.tile([P, 1], F32, tag="neg_big")
                nc.vector.memset(neg_big[:], -BIG)

            sl = 0
            for rg in range(N_RG):
                r0 = rg * P
                for c0, fw in _chunks_for(rg):
                    xt = io_pool.tile([P, fw], F32, tag="x")
                    tt = io_pool.tile([P, fw], I32, tag="t")
                    nc.sync.dma_start(xt[:], x[r0 : r0 + P, c0 : c0 + fw])
                    nc.sync.dma_start(tt[:], t[r0 : r0 + P, c0 : c0 + fw])
                    if dma_only:
                        sl += 1
                        continue

                    ut = u_pool.tile([P, fw], F32, tag="u")
                    # u = C*t - x ; accum -> su = C*n_pos - sum(x)
                    nc.vector.scalar_tensor_tensor(
                        ut[:],
                        tt[:],
                        BIG,
                        xt[:],
                        op0=ALU.mult,
                        op1=ALU.subtract,
                        accum_out=acc_su[:, sl : sl + 1],
                    )
                    ea = e_pool.tile([P, fw], F32, tag="escr")
                    # exp(u - C): t=1 -> exp(-x); t=0 -> 0
                    nc.scalar.activation(
                        ea[:],
                        ut[:],
                        AF.Exp,
                        bias=neg_big[:],
                        scale=1.0,
                        accum_out=acc_spos[:, sl : sl + 1],
                    )
                    eb = e_pool.tile([P, fw], F32, tag="escr")
                    # exp(-u): t=0 -> exp(x); t=1 -> 0
                    nc.scalar.activation(
                        eb[:],
                        ut[:],
                        AF.Exp,
                        scale=-1.0,
                        accum_out=acc_sneg[:, sl : sl + 1],
                    )
                    sl += 1

            if dma_only:
                for a in (acc_spos, acc_sneg, acc_su):
                    nc.vector.memset(a[:, 0:1], 0.0)
            # su completes at the last stt (before the last exps), so its DMA
            # overlaps ACT's tail; spos/sneg DMAs follow their final read-acc.
            nc.sync.dma_start(out[:, 2 * N_SLOTS : 3 * N_SLOTS], acc_su[:])
            nc.sync.dma_start(out[:, 0:N_SLOTS], acc_spos[:])
            nc.sync.dma_start(out[:, N_SLOTS : 2 * N_SLOTS], acc_sneg[:])

    nc.compile()
    return nc


_NC_CACHE = {}


def _get_nc():
    if "nc" not in _NC_CACHE:
        _NC_CACHE["nc"] = build_bass()
    return _NC_CACHE["nc"]


def _shard_inputs(x, t):
    return [
        {
            "x": np.ascontiguousarray(x[i * ROWS : (i + 1) * ROWS]),
            "t": np.ascontiguousarray(t[i * ROWS : (i + 1) * ROWS]),
        }
        for i in range(N_CORES)
    ]


# slot -> row group, precomputed once
_SLOT_RG = np.array(
    [rg for rg in range(N_RG) for _ in _chunks_for(rg)], dtype=np.int64
)


def _combine(core_outs):
    """Host epilogue: core_outs is a list of [P, 3*N_SLOTS] f32 arrays."""
    total = 0.0
    for o in core_outs:
        o = np.asarray(o, dtype=np.float64)
        spos_sl = o[:, 0:N_SLOTS]
        sneg_sl = o[:, N_SLOTS : 2 * N_SLOTS]
        su_sl = o[:, 2 * N_SLOTS : 3 * N_SLOTS]
        # per-row sums over this row group's slots: result [P, N_RG]
        s_pos = np.stack(
            [spos_sl[:, _SLOT_RG == rg].sum(axis=1) for rg in range(N_RG)], axis=1
        )
        s_neg = np.stack(
            [sneg_sl[:, _SLOT_RG == rg].sum(axis=1) for rg in range(N_RG)], axis=1
        )
        su = np.stack(
            [su_sl[:, _SLOT_RG == rg].sum(axis=1) for rg in range(N_RG)], axis=1
        )
        n_pos = np.rint(su / BIG)  # exact: |sum(x) + eps| << BIG/2
        n_neg = float(L) - n_pos
        total += float((s_pos * s_neg / (n_pos * n_neg)).sum())
    return np.float32(total)


def kernel(input, target):
    x = np.ascontiguousarray(np.asarray(input, dtype=np.float32))
    t = np.ascontiguousarray(np.asarray(target, dtype=np.int32))
    assert x.shape == (B, L) and t.shape == (B, L)

    nc = _get_nc()
    res = run_bass_kernel_spmd(
        nc, _shard_inputs(x, t), core_ids=list(range(N_CORES))
    )
    return _combine([res.results[i]["out"] for i in range(N_CORES)])
